# revision 3
# baseline (speedup 1.0000x reference)
"""KernelPoolingLayer (KNRM Gaussian kernel pooling) on 8 trn2 cores — v5.

Math per output [l, b, k]:
  out = sum_q oov[b,q] * 0.01 * log(clip(sum_d m[b,q,d]*exp(-(x-mu_k)^2/(2 s_k^2)), 1e-10))
  mu = [1.0, 0.9, 0.7, ..., -0.9]  (K=11), sigma = [0.001, 0.1, ..., 0.1]

v5 fast path (masks all ones AND x >= 0), per core (rows=L*Bc*Q=1024, D=1024):
  - k=10 is provably clipped: S_10 <= 1024*e^-40.5 < 1e-10, so its column is
    memset to 0 (-> clip -> ln 1e-10) and its chain link dropped.
  - Exact geometric chain E_{k+1} = E_k * R * e^{-4(k-1)}, R = exp(-20x+16),
    fused multiply+D-sum (scalar_tensor_tensor accum_out) links k=2..9 split
    across gpsimd/DVE with per-tile alternation (cuts pipeline fill).
  - ACT per tile PAIR: one [128,2048] Square and one [128,2048] Exp for
    sq1/R (amortizes ACT access latency), per-tile Exp+acc for E1/E0.
  - k=0: (x-1)^2 = sq1 - 0.2x + 0.19 via one gpsimd op pair-wide; +0.19
    folded into E0's Exp bias. E0 emission deferred one tile.
  - Stats: clip/ln once at the end; oov weights folded into the PE q-sum
    matmul rhs (no elementwise oov multiply).
  All chain tensors bf16 (f32 accumulation); sums/log exact to ~1e-4 rel.
"""

import numpy as np

L, B, Q, D = 2, 64, 64, 1024
NCORES = 8
Bc = B // NCORES            # 8
ROWS = L * Bc * Q           # 1024 rows per core
P = 128                     # partitions
NT = ROWS // P              # 8 tiles per core
NPAIR = NT // 2
K = 11
SC = NT * K                 # 88 stats columns
AUXC = 2

MU = [1.0] + [0.9 - 0.2 * (k - 1) for k in range(1, K)]
# chain links k=2..9; alternate which engine leads per tile.  gpsimd stt
# is cheaper (853 vs 1127 ns) so it gets 5 links on gps-first tiles.
GPS_FIRST = ((2, 3, 4, 5, 9), None)   # (gpsimd ks, rest on DVE)
DVE_FIRST = ((6, 7, 8, 9), None)


def _build_aux():
    aux = np.zeros((P, AUXC), np.float32)
    aux[:64, 0] = 1.0
    aux[64:, 1] = 1.0
    return aux


def _build_ovg(ov_core):
    """[P, 2*NT] matmul rhs folding oov weights into the q-sum:
    col 2t+g = 0.01*ov[row(p,t)] for partition-half g, else 0."""
    rowsel = (np.arange(P)[:, None] + P * np.arange(NT)[None, :]) % (Bc * Q)
    w = 0.01 * ov_core[rowsel]
    ovg = np.zeros((P, 2 * NT), np.float32)
    for t in range(NT):
        ovg[:64, 2 * t] = w[:64, t]
        ovg[64:, 2 * t + 1] = w[64:, t]
    return np.ascontiguousarray(ovg)


_CACHE = {}
LAST_RESULT = None
TRACE = False


def _get_built(fast):
    if fast in _CACHE:
        return _CACHE[fast]

    from contextlib import ExitStack
    import concourse.bacc as bacc
    import concourse.mybir as mybir
    import concourse.tile as tile

    f32 = mybir.dt.float32
    bf16 = mybir.dt.bfloat16
    AF = mybir.ActivationFunctionType
    OP = mybir.AluOpType

    nc = bacc.Bacc(
        "TRN2", target_bir_lowering=False, debug=False, num_devices=NCORES
    )
    x_d = nc.dram_tensor("x", [ROWS, D], f32, kind="ExternalInput").ap()
    if fast:
        ovg_d = nc.dram_tensor("ovg", [P, 2 * NT], f32,
                               kind="ExternalInput").ap()
    else:
        ov_d = nc.dram_tensor("ov", [P, SC], f32, kind="ExternalInput").ap()
        aux_d = nc.dram_tensor("aux", [P, AUXC], f32,
                               kind="ExternalInput").ap()
        m_d = nc.dram_tensor("m", [Bc * Q, D], f32, kind="ExternalInput").ap()
    o_d = nc.dram_tensor("o", [K, 2 * NT], f32, kind="ExternalOutput").ap()

    with tile.TileContext(nc) as tc, ExitStack() as ctx:
        xin = ctx.enter_context(tc.tile_pool(name="xin", bufs=2))
        wk = ctx.enter_context(tc.tile_pool(name="wk", bufs=2))
        gp = ctx.enter_context(tc.tile_pool(name="gp", bufs=2))
        singles = ctx.enter_context(tc.tile_pool(name="singles", bufs=1))
        psum = ctx.enter_context(tc.tile_pool(name="psum", bufs=1, space="PSUM"))

        S = singles.tile([P, SC], f32)
        if fast:
            ovgt = singles.tile([P, 2 * NT], f32)
        else:
            auxt = singles.tile([P, AUXC], f32)
            nc.sync.dma_start(out=auxt, in_=aux_d)
            ovt = singles.tile([P, SC], f32)
            nc.sync.dma_start(out=ovt, in_=ov_d)
            ONES2 = auxt[:, 0:2]
            mts = []
            for j in range(Bc * Q // P):
                mt = singles.tile([P, D], f32, tag=f"m{j}")
                nc.sync.dma_start(out=mt, in_=m_d[j * P:(j + 1) * P, :])
                mts.append(mt)

        consts = {}

        def c_ap(v):
            v = float(v)
            if v not in consts:
                t = singles.tile([P, 1], f32, tag=f"cst{len(consts)}")
                nc.vector.memset(t, v)
                consts[v] = t
            return consts[v]

        if fast:
            col = lambda t, k: S[:, t * K + k:t * K + k + 1]
            # warmup Square on a const: hoists the exp/square act-table
            # load to t=0 with no DMA dependencies
            warm = singles.tile([P, 1], f32, tag="warm")
            nc.scalar.activation(warm, c_ap(1.0), AF.Square)
            # k=10 columns: always below the 1e-10 clip -> memset once
            nc.vector.memset(S[:, 10::K], 0.0)
            pend = []

            def emit_e0(tt, t0c):
                E0 = wk.tile([P, D], bf16, tag="e0")
                nc.scalar.activation(E0, t0c, AF.Exp, scale=c_ap(-500000.0),
                                     bias=c_ap(-95000.0),
                                     accum_out=col(tt, 0))

            def emit_chain(t, E1, Rc):
                gps_ks, _ = GPS_FIRST if t % 2 == 0 else DVE_FIRST
                G = E1
                for k in range(2, K - 1):
                    eng = nc.gpsimd if k in gps_ks else nc.vector
                    Gn = gp.tile([P, D], bf16, tag=f"g{k}")
                    eng.scalar_tensor_tensor(
                        out=Gn, in0=G, scalar=float(np.exp(-4.0 * (k - 2))),
                        in1=Rc, op0=OP.mult, op1=OP.mult,
                        accum_out=col(t, k))
                    G = Gn

            def emit_chain_split(t, E1, Rc, gps_even):
                """Two parallel sub-chains via R^2: serial depth 5 not 8.
                even: G2->G4->G6->G8, odd: G3->G5->G7->G9."""
                R2 = gp.tile([P, D], bf16, tag="r2")
                nc.vector.tensor_mul(R2, Rc, Rc)
                ge = nc.gpsimd if gps_even else nc.vector
                go = nc.vector if gps_even else nc.gpsimd
                Gs = {}
                G2 = gp.tile([P, D], bf16, tag="g2")
                ge.scalar_tensor_tensor(
                    out=G2, in0=E1, scalar=1.0, in1=Rc,
                    op0=OP.mult, op1=OP.mult, accum_out=col(t, 2))
                G3 = gp.tile([P, D], bf16, tag="g3")
                go.scalar_tensor_tensor(
                    out=G3, in0=G2, scalar=float(np.exp(-4.0)), in1=Rc,
                    op0=OP.mult, op1=OP.mult, accum_out=col(t, 3))
                Gs[2], Gs[3] = G2, G3
                for k in range(4, K - 1):
                    eng = ge if k % 2 == 0 else go
                    Gn = gp.tile([P, D], bf16, tag=f"g{k}")
                    # E_k = E_{k-2} * R^2 * e^{-8(k-2)+4}
                    eng.scalar_tensor_tensor(
                        out=Gn, in0=Gs[k - 2],
                        scalar=float(np.exp(-8.0 * (k - 2) + 4.0)),
                        in1=R2, op0=OP.mult, op1=OP.mult,
                        accum_out=col(t, k))
                    Gs[k] = Gn

            def emit_seeds(t, xt, sq, Rc, E1):
                nc.scalar.activation(sq, xt, AF.Square, bias=c_ap(-MU[1]))
                nc.scalar.activation(E1, sq, AF.Exp,
                                     scale=c_ap(-50.0), accum_out=col(t, 1))
                nc.scalar.activation(Rc, xt, AF.Exp, scale=c_ap(-20.0),
                                     bias=c_ap(16.0))

            # tiles 0,1 unpaired (fill): ACT starts after one DMA; links
            # ping-pong across engines so both chain engines engage early.
            for t in (0, 1):
                xt = xin.tile([P, D], f32, tag="xs")
                nc.sync.dma_start(out=xt, in_=x_d[t * P:(t + 1) * P, :])
                sq = wk.tile([P, D], f32, tag="sqs")
                E1 = wk.tile([P, D], bf16, tag="e1s")
                Rc = wk.tile([P, D], bf16, tag="rs")
                emit_seeds(t, xt, sq, Rc, E1)
                t0c = wk.tile([P, D], f32, tag="t0s")
                nc.gpsimd.scalar_tensor_tensor(
                    out=t0c, in0=xt, scalar=-0.2, in1=sq,
                    op0=OP.mult, op1=OP.add)
                emit_chain_split(t, E1, Rc, gps_even=(t % 2 == 0))
                pend.append((t, t0c))

            # middle tiles paired (amortized ACT ops)
            for pr in range(1, NPAIR - 1):
                ta, tb = 2 * pr, 2 * pr + 1
                xp = xin.tile([P, 2 * D], f32, tag="x")
                nc.sync.dma_start(out=xp[:, 0:D],
                                  in_=x_d[ta * P:(ta + 1) * P, :])
                nc.sync.dma_start(out=xp[:, D:2 * D],
                                  in_=x_d[tb * P:(tb + 1) * P, :])

                sqp = wk.tile([P, 2 * D], f32, tag="sqp")
                nc.scalar.activation(sqp, xp, AF.Square, bias=c_ap(-MU[1]))
                E1a = wk.tile([P, D], bf16, tag="e1a")
                nc.scalar.activation(E1a, sqp[:, 0:D], AF.Exp,
                                     scale=c_ap(-50.0), accum_out=col(ta, 1))
                Rp = wk.tile([P, 2 * D], bf16, tag="rp")
                nc.scalar.activation(Rp, xp, AF.Exp, scale=c_ap(-20.0),
                                     bias=c_ap(16.0))
                E1b = wk.tile([P, D], bf16, tag="e1b")
                nc.scalar.activation(E1b, sqp[:, D:2 * D], AF.Exp,
                                     scale=c_ap(-50.0), accum_out=col(tb, 1))

                t0p = wk.tile([P, 2 * D], f32, tag="t0p", bufs=3)
                nc.gpsimd.scalar_tensor_tensor(
                    out=t0p, in0=xp, scalar=-0.2, in1=sqp,
                    op0=OP.mult, op1=OP.add)

                emit_chain(ta, E1a, Rp[:, 0:D])
                emit_chain(tb, E1b, Rp[:, D:2 * D])

                pend.append((ta, t0p[:, 0:D]))
                pend.append((tb, t0p[:, D:2 * D]))
                while len(pend) > 3:
                    emit_e0(*pend.pop(0))

            # last two tiles unpaired with split chains (short drain)
            for t in (NT - 2, NT - 1):
                xt = xin.tile([P, D], f32, tag="xs")
                nc.sync.dma_start(out=xt, in_=x_d[t * P:(t + 1) * P, :])
                sq = wk.tile([P, D], f32, tag="sqs")
                E1 = wk.tile([P, D], bf16, tag="e1s")
                Rc = wk.tile([P, D], bf16, tag="rs")
                emit_seeds(t, xt, sq, Rc, E1)
                t0c = wk.tile([P, D], f32, tag="t0s")
                nc.gpsimd.scalar_tensor_tensor(
                    out=t0c, in0=xt, scalar=-0.2, in1=sq,
                    op0=OP.mult, op1=OP.add)
                emit_chain_split(t, E1, Rc, gps_even=(t % 2 == 0))
                pend.append((t, t0c))
            while pend:
                emit_e0(*pend.pop(0))
            nc.sync.dma_start(out=ovgt, in_=ovg_d)
        else:
            for t in range(NT):
                xt = xin.tile([P, D], f32, tag="x")
                nc.sync.dma_start(out=xt, in_=x_d[t * P:(t + 1) * P, :])
                col1 = lambda k: S[:, t * K + k:t * K + k + 1]

                sq = wk.tile([P, D], f32, tag="sq1")
                nc.scalar.activation(sq, xt, AF.Square, bias=c_ap(-MU[1]))
                E1 = wk.tile([P, D], f32, tag="e1f")
                nc.scalar.activation(E1, sq, AF.Exp, scale=c_ap(-50.0))
                R = wk.tile([P, D], f32, tag="rf")
                nc.scalar.activation(R, xt, AF.Exp, scale=c_ap(-20.0),
                                     bias=c_ap(16.0))
                sq0 = wk.tile([P, D], f32, tag="sq0")
                nc.scalar.activation(sq0, xt, AF.Square, bias=c_ap(-MU[0]))
                E0 = wk.tile([P, D], f32, tag="e0f")
                nc.scalar.activation(E0, sq0, AF.Exp, scale=c_ap(-500000.0))

                mt = mts[t % len(mts)]
                E1m = gp.tile([P, D], f32, tag="gg")
                nc.vector.scalar_tensor_tensor(
                    out=E1m, in0=E1, scalar=1.0, in1=mt,
                    op0=OP.mult, op1=OP.mult, accum_out=col1(1))
                E0m = wk.tile([P, D], f32, tag="e0m")
                nc.vector.scalar_tensor_tensor(
                    out=E0m, in0=E0, scalar=1.0, in1=mt,
                    op0=OP.mult, op1=OP.mult, accum_out=col1(0))
                G = E1m
                for k in range(2, K):
                    Gn = gp.tile([P, D], f32, tag="gg")
                    nc.vector.scalar_tensor_tensor(
                        out=Gn, in0=G, scalar=float(np.exp(-4.0 * (k - 2))),
                        in1=R, op0=OP.mult, op1=OP.mult, accum_out=col1(k))
                    G = Gn

        # --- stats: clip/log then oov-weighted q-sum via PE ---
        U = singles.tile([P, SC], f32)
        nc.vector.tensor_scalar_max(U, S, 1e-10)
        LG = singles.tile([P, SC], f32)
        nc.scalar.activation(LG, U, AF.Ln)
        if not fast:
            V = singles.tile([P, SC], f32)
            nc.vector.tensor_mul(V, LG, ovt)
            LG = V

        ps = psum.tile([P, 2 * NT], f32)
        for t in range(NT):
            rhs = ovgt[:, 2 * t:2 * t + 2] if fast else ONES2
            nc.tensor.matmul(
                out=ps[0:K, 2 * t:2 * t + 2],
                lhsT=LG[:, t * K:(t + 1) * K], rhs=rhs,
                start=True, stop=True)
        OT = singles.tile([P, 2 * NT], f32)
        nc.vector.tensor_copy(OT[0:K, :], ps[0:K, :])
        nc.sync.dma_start(out=o_d, in_=OT[0:K, :])

    nc.compile()
    _CACHE[fast] = nc
    return nc


def _in_maps(match_matrices, query_by_doc_mask, query_pad_oov_mask):
    x = np.ascontiguousarray(np.asarray(match_matrices, dtype=np.float32))
    m = np.ascontiguousarray(np.asarray(query_by_doc_mask, dtype=np.float32))
    ov = np.ascontiguousarray(np.asarray(query_pad_oov_mask, dtype=np.float32))
    fast = bool((m == 1.0).all()) and bool((x >= 0.0).all())
    aux = _build_aux()
    rowsel = (np.arange(P)[:, None] + P * np.arange(NT)[None, :]) % (Bc * Q)
    in_maps = []
    for c in range(NCORES):
        xs = x[:, c * Bc:(c + 1) * Bc].reshape(ROWS, D)
        ovs = ov[c * Bc:(c + 1) * Bc].reshape(Bc * Q).astype(np.float32)
        if fast:
            im = {"x": xs, "ovg": _build_ovg(ovs)}
        else:
            OV = np.repeat((0.01 * ovs[rowsel]).astype(np.float32), K, axis=1)
            im = {"x": xs, "ov": np.ascontiguousarray(OV), "aux": aux,
                  "m": np.ascontiguousarray(
                      m[c * Bc:(c + 1) * Bc].reshape(Bc * Q, D))}
        in_maps.append(im)
    return fast, in_maps


def simulate(match_matrices, query_by_doc_mask, query_pad_oov_mask):
    """CoreSim all 8 cores: returns (full output, max sim ns)."""
    from concourse.bass_interp import CoreSim

    fast, in_maps = _in_maps(
        match_matrices, query_by_doc_mask, query_pad_oov_mask)
    nc = _get_built(fast)
    outs, t = [], 0.0
    for c in range(NCORES):
        sim = CoreSim(nc)
        for name, val in in_maps[c].items():
            sim.tensor(name)[:] = val
        sim.simulate()
        outs.append(np.array(sim.tensor("o")).T.reshape(L, Bc, K))
        t = max(t, sim.time)
    return np.concatenate(outs, axis=1), t


def kernel(match_matrices, query_by_doc_mask, query_pad_oov_mask):
    global LAST_RESULT
    from concourse.bass_utils import run_bass_kernel_spmd

    fast, in_maps = _in_maps(
        match_matrices, query_by_doc_mask, query_pad_oov_mask)
    nc = _get_built(fast)
    LAST_RESULT = run_bass_kernel_spmd(
        nc, in_maps, core_ids=list(range(NCORES)), trace=TRACE)
    outs = [LAST_RESULT.results[c]["o"].T.reshape(L, Bc, K)
            for c in range(NCORES)]
    return np.concatenate(outs, axis=1)
